# revision 1
# baseline (speedup 1.0000x reference)
"""DigitCaps dynamic-routing kernel for 8 Trainium2 NeuronCores.

Sharding: J (num_capsule=32) split 8 ways -> 4 capsules per core, batch
replicated. W is SBUF-resident in its natural layout for the i-contraction
GEMMs; the transposed layout is streamed for the p-contraction routing
matmuls. The routing softmax over J uses a cross-core AllReduce of
per-(b,i) partial exp sums; a renormalization-invariance trick keeps a
single running tensor F (= c, up to a shared normalizer) instead of exp(b).

Per core (j = 4 local capsules, B=64, I=2048, Q=16, P=32):
  hat[b,j,i,p] = sum_q x[b,i,q] W[j,i,p,q]       (never materialized)
  v1 = squash(S/32),  S = sum_{i,q} x W          (c1 uniform)
  Delta_k[b,j,i] = sum_q x[b,i,q] * (Wt^T vbd_k)[b,j,(i,q)]
  F <- F * exp(Delta);  Z' = AllReduce_j(sum_j F);  F <- F / Z'   (= c)
  v_k = squash(sum_{i,q} (F x) W)
  out = v3
"""

import numpy as np
import ml_dtypes

import concourse.bacc as bacc
import concourse.mybir as mybir
import concourse.tile as tile
from concourse.bass_utils import run_bass_kernel_spmd
from concourse.masks import make_identity

BF16 = mybir.dt.bfloat16
F32 = mybir.dt.float32
NP_BF16 = ml_dtypes.bfloat16

N_CORES = 8
B = 64
I = 2048
Q = 16
J = 32
P = 32
JL = J // N_CORES
ICH = I // 128
EPS = 1e-7
AF = mybir.ActivationFunctionType

_CACHED = {}


def _squash(nc, small, v_sb, eps_ap):
    """In-place squash over p of v_sb [64, JL*P] fp32 (free = (j, p))."""
    sq = small.tile([B, JL * P], F32, tag="sq")
    nc.vector.tensor_mul(sq[:], v_sb[:], v_sb[:])
    red = sq.rearrange("b (j p) -> b j p", j=JL)
    w = P
    while w > 1:
        h = w // 2
        nc.vector.tensor_add(red[:, :, 0:h], red[:, :, 0:h], red[:, :, h:w])
        w = h
    s2 = small.tile([B, JL], F32, tag="s2")
    nc.vector.tensor_copy(s2[:], red[:, :, 0])
    rt = small.tile([B, JL], F32, tag="rt")
    nc.scalar.activation(rt[:], s2[:], AF.Sqrt, bias=eps_ap[:B, :])
    den = small.tile([B, JL], F32, tag="den")
    nc.vector.tensor_mul(den[:], s2[:], rt[:])
    nc.vector.tensor_add(den[:], den[:], rt[:])
    rec = small.tile([B, JL], F32, tag="rec")
    nc.vector.reciprocal(rec[:], den[:])
    scale = small.tile([B, JL], F32, tag="scale")
    nc.vector.tensor_mul(scale[:], s2[:], rec[:])
    vv = v_sb.rearrange("b (j p) -> b j p", j=JL)
    sc_b = scale.unsqueeze(2).broadcast_to([B, JL, P])
    nc.vector.tensor_mul(vv[:], vv[:], sc_b[:])


def _build_vbd(nc, small, psum_t, v_sb, identity):
    """v_sb [64, (j,p)] fp32 -> two block-diag bf16 lhsT [128, (jj 2, b 64)]."""
    vt_ps = psum_t.tile([128, B], F32, tag="vt_ps")
    nc.tensor.transpose(vt_ps[:], v_sb[:], identity[:B, :B])
    vt = small.tile([128, B], F32, tag="vt")
    nc.scalar.copy(vt[:], vt_ps[:])  # [(j,p), b]
    vbds = []
    for pair in range(2):
        vbd = small.tile([128, 2 * B], BF16, tag=f"vbd{pair}")
        nc.vector.memset(vbd[:], 0.0)
        for jj in range(2):
            j = pair * 2 + jj
            nc.vector.tensor_copy(
                vbd[j * P:(j + 1) * P, jj * B:(jj + 1) * B],
                vt[j * P:(j + 1) * P, :],
            )
        vbds.append(vbd)
    return vbds


def _vT_to_v(nc, small, ps_vt, vT_ps, identity, scale=None):
    """vT psum [128 (j,p), 64 b] -> v_sb [64, (j,p)] fp32 via evac+transpose."""
    vT = small.tile([128, B], F32, tag="vTe")
    if scale is None:
        nc.scalar.copy(vT[:], vT_ps[:])
    else:
        nc.scalar.mul(vT[:], vT_ps[:], scale)
    v_ps = ps_vt.tile([B, 128], F32, tag="v_ps2")
    nc.tensor.transpose(v_ps[:], vT[:], identity[:])
    v_sb = small.tile([B, JL * P], F32, tag="v")
    nc.scalar.copy(v_sb[:], v_ps[:])
    return v_sb


def build_kernel():
    if "nc" in _CACHED:
        return _CACHED["nc"]
    nc = bacc.Bacc(
        "TRN2", target_bir_lowering=False, debug=False, num_devices=N_CORES
    )
    wn_d = nc.dram_tensor("wn", [128, ICH * Q * JL * P], BF16, kind="ExternalInput")
    wt_d = nc.dram_tensor("wt", [128, I * Q], BF16, kind="ExternalInput")
    xq_d = nc.dram_tensor("xq", [128, ICH * Q * B], BF16, kind="ExternalInput")
    xt_d = nc.dram_tensor("xt", [128, I * Q], BF16, kind="ExternalInput")
    out_d = nc.dram_tensor("o", [B, JL * P], F32, kind="ExternalOutput")

    with tile.TileContext(nc) as tc:
        with (
            tc.tile_pool(name="big", bufs=1) as big,
            tc.tile_pool(name="wts", bufs=2) as wts,
            tc.tile_pool(name="ustr", bufs=2) as ustr,
            tc.tile_pool(name="ustr_t", bufs=1) as ustr_t,
            tc.tile_pool(name="estr", bufs=2) as estr,
            tc.tile_pool(name="small", bufs=1) as small,
            tc.tile_pool(name="ytile", bufs=4) as ytile,
            tc.tile_pool(name="dram", bufs=4, space="DRAM") as dram,
        ):
            # ---- resident loads -------------------------------------
            wn = big.tile([128, ICH * Q * JL * P], BF16, tag="wn")   # 64K/part
            nc.sync.dma_start(wn[:], wn_d[:])
            wnv = wn.rearrange("k (ich q j p) -> k ich q j p", ich=ICH, q=Q, j=JL)
            xq = big.tile([128, ICH * Q * B], BF16, tag="xq")        # 32K/part
            nc.sync.dma_start(xq[:], xq_d[:])
            xqv = xq.rearrange("k (ich q b) -> k ich q b", ich=ICH, q=Q)
            xt = big.tile([128, I * Q], BF16, tag="xt")              # 64K/part
            nc.sync.dma_start(xt[:], xt_d[:])

            identity = big.tile([128, 128], F32, tag="ident")
            make_identity(nc, identity[:])
            identb = big.tile([128, 128], BF16, tag="identb")
            make_identity(nc, identb[:])
            eps_t = big.tile([128, 1], F32, tag="eps")
            nc.vector.memset(eps_t[:], EPS)

            # F[ip, (ich, j, b)] bf16: running c (up to global normalizer)
            f_sb = big.tile([128, ICH * JL * B], BF16, tag="f")      # 8K/part
            f_v = f_sb.rearrange("k (ich j b) -> k ich j b", ich=ICH, j=JL)

            # warmup collective to absorb core-start skew
            wu_s = small.tile([128, 8], F32, tag="wu")
            nc.gpsimd.memset(wu_s[:], 0.0)
            wu_i = dram.tile([128, 8], F32, tag="wu_i")
            wu_o = dram.tile([128, 8], F32, tag="wu_o")
            nc.gpsimd.dma_start(wu_i[:], wu_s[:])
            nc.gpsimd.collective_compute(
                "AllReduce", mybir.AluOpType.add,
                replica_groups=[list(range(N_CORES))],
                ins=[wu_i.opt()], outs=[wu_o.opt()],
            )

            # ---- S-pass: vT[(j,p), b] = sum_{i,q} W x ---------------
            with tc.tile_pool(name="ps_s", bufs=1, space="PSUM") as ps_s, \
                 tc.tile_pool(name="ps_st", bufs=1, space="PSUM") as ps_st:
                s_ps = ps_s.tile([128, B], F32, tag="s_ps")
                n_mm = ICH * Q
                k = 0
                for ich in range(ICH):
                    for q in range(Q):
                        nc.tensor.matmul(
                            s_ps[:],
                            wnv[:, ich, q, :, :],       # lhsT [128, (j p)]
                            xqv[:, ich, q, :],          # rhs  [128, 64]
                            start=(k == 0), stop=(k == n_mm - 1),
                        )
                        k += 1
                v_sb = _vT_to_v(nc, small, ps_st, s_ps, identity, scale=1.0 / J)
                _squash(nc, small, v_sb, eps_t)
                vbds = _build_vbd(nc, small, ps_st, v_sb, identity)

            # ---- 2 routing iterations -------------------------------
            for it in range(2):
                first = it == 0
                # b-pass
                cc_pend = [None, None]
                with tc.tile_pool(name=f"ps_b{it}", bufs=1, space="PSUM") as ps_b, \
                     tc.tile_pool(name=f"ps_bt{it}", bufs=2, space="PSUM") as ps_bt:
                    for g in range(ICH):
                        wt_s = wts.tile([128, 128 * Q], BF16, tag="wt_s")
                        nc.sync.dma_start(
                            wt_s[:], wt_d[:, g * 128 * Q:(g + 1) * 128 * Q]
                        )
                        for pair in range(2):
                            dwin = estr.tile(
                                [128, 128], BF16, tag="dwin",
                                name=f"dwin{it}_{g}_{pair}",
                            )
                            t_ps = ps_b.tile(
                                [128, 128 * Q], F32, tag="t_ps",
                                name=f"t_ps{it}_{g}_{pair}",
                            )
                            for m in range(4):
                                nc.tensor.matmul(
                                    t_ps[:, m * 512:(m + 1) * 512],
                                    vbds[pair][:],
                                    wt_s[:, m * 512:(m + 1) * 512],
                                    start=True, stop=True,
                                )
                            u = ustr.tile(
                                [128, 128 * Q], BF16, tag="u",
                                name=f"u{it}_{g}_{pair}",
                            )
                            xoff = g * 2048
                            if pair == 0:
                                # ScalarE evacuates, DVE multiplies in bf16 2x
                                t_sb = ustr_t.tile(
                                    [128, 128 * Q], BF16, tag="t_sb",
                                    name=f"t_sb{it}_{g}",
                                )
                                for m in range(4):
                                    nc.scalar.copy(
                                        t_sb[:, m * 512:(m + 1) * 512],
                                        t_ps[:, m * 512:(m + 1) * 512],
                                    )
                                nc.vector.tensor_mul(
                                    u[:], t_sb[:], xt[:, xoff:xoff + 2048]
                                )
                            else:
                                # DVE multiplies straight out of PSUM (1x)
                                nc.vector.tensor_mul(
                                    u[:], t_ps[:], xt[:, xoff:xoff + 2048]
                                )
                            # contiguous tree-reduce over q -> [128, 128]
                            w = Q
                            while w > 2:
                                h = w // 2
                                nc.vector.tensor_add(
                                    u[:, 0:h * 128], u[:, 0:h * 128],
                                    u[:, h * 128:w * 128],
                                )
                                w = h
                            nc.vector.tensor_add(
                                dwin[:], u[:, 0:128], u[:, 128:256]
                            )
                            d_ps = ps_bt.tile([128, 128], BF16, tag="d_ps")
                            nc.tensor.transpose(d_ps[:], dwin[:], identb[:])
                            off = (g * JL + pair * 2) * B
                            dst = f_sb[:, off:off + 2 * B]
                            if first:
                                nc.scalar.activation(dst, d_ps[:], AF.Exp)
                            else:
                                ex = estr.tile([128, 128], BF16, tag="ex")
                                nc.scalar.activation(ex[:], d_ps[:], AF.Exp)
                                nc.vector.tensor_mul(dst, dst, ex[:])
                        if g == 7 or g == ICH - 1:
                            h = 0 if g == 7 else 1
                            sl = slice(h * 8, h * 8 + 8)
                            zph = small.tile(
                                [128, 8 * B], F32, tag=f"zp{h}",
                                name=f"zp{it}_{h}",
                            )
                            zpv = zph.rearrange("k (ic b) -> k ic b", ic=8)
                            nc.vector.tensor_add(
                                zpv[:], f_v[:, sl, 0, :], f_v[:, sl, 1, :]
                            )
                            for j in range(2, JL):
                                nc.vector.tensor_add(
                                    zpv[:], zpv[:], f_v[:, sl, j, :]
                                )
                            cc_i = dram.tile(
                                [128, 8 * B], F32, tag=f"cc_i{h}",
                                name=f"cci{it}_{h}",
                            )
                            cc_o = dram.tile(
                                [128, 8 * B], F32, tag=f"cc_o{h}",
                                name=f"cco{it}_{h}",
                            )
                            nc.gpsimd.dma_start(cc_i[:], zph[:])
                            nc.gpsimd.collective_compute(
                                "AllReduce", mybir.AluOpType.add,
                                replica_groups=[list(range(N_CORES))],
                                ins=[cc_i.opt()], outs=[cc_o.opt()],
                            )
                            cc_pend[h] = cc_o

                # softmax normalizer across cores, pipelined in i-halves:
                # half-0 AllReduce was issued mid-b-pass (below); issue half-1
                cc_os = [None, None]
                for h in range(2):
                    if cc_pend[h] is not None:
                        cc_os[h] = cc_pend[h]
                # v-pass: vT[(j,p), b] = sum_{i,q} W (F x), col-tiled over j
                with tc.tile_pool(name=f"ps_v{it}", bufs=1, space="PSUM") as ps_v, \
                     tc.tile_pool(name=f"ps_vt{it}", bufs=2, space="PSUM") as ps_vt:
                    vT_ps = ps_v.tile([128, B], F32, tag="vT_ps")
                    for h in range(2):
                        sl = slice(h * 8, h * 8 + 8)
                        zh = small.tile(
                            [128, 8 * B], F32, tag=f"z{h}", name=f"z{it}_{h}"
                        )
                        nc.sync.dma_start(zh[:], cc_os[h][:])
                        nc.vector.reciprocal(zh[:], zh[:])
                        zrv = zh.rearrange("k (ic b) -> k ic b", ic=8)
                        for j in range(JL):
                            nc.vector.tensor_mul(
                                f_v[:, sl, j, :], f_v[:, sl, j, :], zrv[:]
                            )
                    for ich in range(ICH):
                        for qh in range(2):
                            ys = []
                            for j in range(JL):
                                y = ytile.tile(
                                    [128, 8 * B], BF16, tag="y",
                                    name=f"y{it}_{ich}_{qh}_{j}",
                                )
                                yv = y.rearrange("k (q b) -> k q b", q=8)
                                cb = (
                                    f_v[:, ich, j, :]
                                    .unsqueeze(1).broadcast_to([128, 8, B])
                                )
                                nc.vector.tensor_mul(
                                    yv[:],
                                    xqv[:, ich, qh * 8:(qh + 1) * 8, :],
                                    cb[:],
                                )
                                ys.append(yv)
                            for qq in range(8):
                                q = qh * 8 + qq
                                for j in range(JL):
                                    nc.tensor.matmul(
                                        vT_ps[j * P:(j + 1) * P, :],
                                        wnv[:, ich, q, j, :],
                                        ys[j][:, qq, :],
                                        start=(ich == 0 and q == 0),
                                        stop=(ich == ICH - 1 and q == Q - 1),
                                        tile_position=(0, j * P),
                                    )
                    v_sb = _vT_to_v(nc, small, ps_vt, vT_ps, identity)
                    _squash(nc, small, v_sb, eps_t)
                    if it == 0:
                        vbds = _build_vbd(nc, small, ps_vt, v_sb, identity)
                    else:
                        nc.sync.dma_start(out_d[:], v_sb[:])

    nc.compile()
    _CACHED["nc"] = nc
    return nc


def _prep_inputs(inputs_np, W_np):
    x = np.ascontiguousarray(inputs_np)           # [B, I, Q] f32
    W = np.ascontiguousarray(W_np)                # [J, I, P, Q] f32
    xq = (
        x.reshape(B, ICH, 128, Q).transpose(2, 1, 3, 0)
        .astype(NP_BF16).reshape(128, ICH * Q * B)
    )
    # xt cols ordered (g, q, iw): matches wt streaming windows
    xt_base = (
        x.reshape(B, ICH, 128, Q).transpose(0, 1, 3, 2)   # [b, g, q, iw]
        .astype(NP_BF16).reshape(B, I * Q)
    )
    xt = np.concatenate([xt_base, xt_base], axis=0)
    in_maps = []
    for r in range(N_CORES):
        Wr = W[r * JL:(r + 1) * JL]                       # [4, I, P, Q]
        wn = (
            Wr.reshape(JL, ICH, 128, P, Q).transpose(2, 1, 4, 0, 3)
            .astype(NP_BF16).reshape(128, ICH * Q * JL * P)
        )
        wt = (
            Wr.reshape(JL, ICH, 128, P, Q)
            .transpose(0, 3, 1, 4, 2)                     # [j, p, g, q, iw]
            .astype(NP_BF16).reshape(128, I * Q)
        )
        in_maps.append(
            {
                "wn": np.ascontiguousarray(wn),
                "wt": np.ascontiguousarray(wt),
                "xq": np.ascontiguousarray(xq),
                "xt": np.ascontiguousarray(xt),
            }
        )
    return in_maps


def kernel(inputs, W, _trace=False):
    nc = build_kernel()
    in_maps = _prep_inputs(np.asarray(inputs), np.asarray(W))
    res = run_bass_kernel_spmd(nc, in_maps, list(range(N_CORES)), trace=_trace)
    out = np.concatenate(
        [res.results[r]["o"].reshape(B, JL, P) for r in range(N_CORES)], axis=1
    )
    if _trace:
        kernel.last_exec_ns = res.exec_time_ns
        kernel.last_results = res
    return out.astype(np.float32)



# revision 6
# speedup vs baseline: 1.2624x; 1.2624x over previous
"""DigitCaps dynamic-routing kernel for 8 Trainium2 NeuronCores.

Sharding: J (num_capsule=32) split 8 ways -> 4 capsules per core, batch
replicated. W is SBUF-resident in its natural layout for the i-contraction
GEMMs; the transposed layout is streamed for the p-contraction routing
matmuls. The routing softmax over J uses a cross-core AllReduce of
per-(b,i) partial exp sums; a renormalization-invariance trick keeps a
single running tensor F (= c, up to a shared normalizer) instead of exp(b).

Per core (j = 4 local capsules, B=64, I=2048, Q=16, P=32):
  hat[b,j,i,p] = sum_q x[b,i,q] W[j,i,p,q]       (never materialized)
  v1 = squash(S/32),  S = sum_{i,q} x W          (c1 uniform)
  Delta_k[b,j,i] = sum_q x[b,i,q] * (Wt^T vbd_k)[b,j,(i,q)]
  F <- F * exp(Delta);  Z' = AllReduce_j(sum_j F);  F <- F / Z'   (= c)
  v_k = squash(sum_{i,q} (F x) W)
  out = v3
"""

import numpy as np
import ml_dtypes

import concourse.bacc as bacc
import concourse.mybir as mybir
import concourse.tile as tile
from concourse.bass_utils import run_bass_kernel_spmd
from concourse.masks import make_identity

BF16 = mybir.dt.bfloat16
F32 = mybir.dt.float32
NP_BF16 = ml_dtypes.bfloat16

N_CORES = 8
B = 64
I = 2048
Q = 16
J = 32
P = 32
JL = J // N_CORES
ICH = I // 128
EPS = 1e-7
AF = mybir.ActivationFunctionType

# b-pass half-chunks whose PSUM evac goes through ScalarE (the rest are
# multiplied straight out of PSUM at DVE 1x). Of 4 halves per g, evac this
# many on ACT:
EVAC_OF_4 = 3
# fold the last tree level into accumulating PE transposes (bf16 PSUM
# accumulate via transpose). Fallback False = known-good single transpose.
TRANSPOSE_ACC = False

_CACHED = {}


def _squash(nc, small, v_sb, eps_ap):
    """In-place squash over p of v_sb [64, JL*P] fp32 (free = (j, p))."""
    sq = small.tile([B, JL * P], F32, tag="sq")
    nc.vector.tensor_mul(sq[:], v_sb[:], v_sb[:])
    red = sq.rearrange("b (j p) -> b j p", j=JL)
    w = P
    while w > 1:
        h = w // 2
        nc.vector.tensor_add(red[:, :, 0:h], red[:, :, 0:h], red[:, :, h:w])
        w = h
    s2 = small.tile([B, JL], F32, tag="s2")
    nc.vector.tensor_copy(s2[:], red[:, :, 0])
    rt = small.tile([B, JL], F32, tag="rt")
    nc.scalar.activation(rt[:], s2[:], AF.Sqrt, bias=eps_ap[:B, :])
    den = small.tile([B, JL], F32, tag="den")
    nc.vector.tensor_mul(den[:], s2[:], rt[:])
    nc.vector.tensor_add(den[:], den[:], rt[:])
    rec = small.tile([B, JL], F32, tag="rec")
    nc.vector.reciprocal(rec[:], den[:])
    scale = small.tile([B, JL], F32, tag="scale")
    nc.vector.tensor_mul(scale[:], s2[:], rec[:])
    vv = v_sb.rearrange("b (j p) -> b j p", j=JL)
    sc_b = scale.unsqueeze(2).broadcast_to([B, JL, P])
    nc.vector.tensor_mul(vv[:], vv[:], sc_b[:])


def _build_vbd(nc, small, psum_t, v_sb, identity):
    """v_sb [64, (j,p)] fp32 -> two block-diag bf16 lhsT [128, (jj 2, b 64)]."""
    vt_ps = psum_t.tile([128, B], F32, tag="vt_ps")
    nc.tensor.transpose(vt_ps[:], v_sb[:], identity[:B, :B])
    vt = small.tile([128, B], F32, tag="vt")
    nc.scalar.copy(vt[:], vt_ps[:])  # [(j,p), b]
    vbds = []
    for pair in range(2):
        vbd = small.tile([128, 2 * B], BF16, tag=f"vbd{pair}")
        nc.vector.memset(vbd[:], 0.0)
        for jj in range(2):
            j = pair * 2 + jj
            nc.vector.tensor_copy(
                vbd[j * P:(j + 1) * P, jj * B:(jj + 1) * B],
                vt[j * P:(j + 1) * P, :],
            )
        vbds.append(vbd)
    return vbds


def _vT_to_v(nc, small, ps_vt, vT_ps, identity, scale=None):
    """vT psum [128 (j,p), 64 b] -> v_sb [64, (j,p)] fp32 via evac+transpose."""
    vT = small.tile([128, B], F32, tag="vTe")
    if scale is None:
        nc.scalar.copy(vT[:], vT_ps[:])
    else:
        nc.scalar.mul(vT[:], vT_ps[:], scale)
    v_ps = ps_vt.tile([B, 128], F32, tag="v_ps2")
    nc.tensor.transpose(v_ps[:], vT[:], identity[:])
    v_sb = small.tile([B, JL * P], F32, tag="v")
    nc.scalar.copy(v_sb[:], v_ps[:])
    return v_sb


def build_kernel():
    if "nc" in _CACHED:
        return _CACHED["nc"]
    nc = bacc.Bacc(
        "TRN2", target_bir_lowering=False, debug=False, num_devices=N_CORES
    )
    wn_d = nc.dram_tensor("wn", [128, ICH * Q * JL * P], BF16, kind="ExternalInput")
    wt_d = nc.dram_tensor("wt", [128, I * Q], BF16, kind="ExternalInput")
    xq_d = nc.dram_tensor("xq", [128, ICH * Q * B], BF16, kind="ExternalInput")
    xt_d = nc.dram_tensor("xt", [128, I * Q], BF16, kind="ExternalInput")
    out_d = nc.dram_tensor("o", [B, JL * P], F32, kind="ExternalOutput")

    with tile.TileContext(nc) as tc:
        with (
            tc.tile_pool(name="big", bufs=1) as big,
            tc.tile_pool(name="wts", bufs=3) as wts,
            tc.tile_pool(name="tstr", bufs=4) as tstr,
            tc.tile_pool(name="estr", bufs=2) as estr,
            tc.tile_pool(name="small", bufs=1) as small,
            tc.tile_pool(name="ytile", bufs=5) as ytile,
            tc.tile_pool(name="dram", bufs=4, space="DRAM") as dram,
        ):
            # ---- resident loads (chunked so consumers start early) ----
            wn = big.tile([128, ICH * Q * JL * P], BF16, tag="wn")   # 64K/part
            wn_chunk = ICH * Q * JL * P // 4
            for c in range(4):
                nc.sync.dma_start(
                    wn[:, c * wn_chunk:(c + 1) * wn_chunk],
                    wn_d[:, c * wn_chunk:(c + 1) * wn_chunk],
                )
            wnv = wn.rearrange("k (ich q j p) -> k ich q j p", ich=ICH, q=Q, j=JL)
            xq = big.tile([128, ICH * Q * B], BF16, tag="xq")        # 32K/part
            xq_chunk = ICH * Q * B // 2
            for c in range(2):
                nc.sync.dma_start(
                    xq[:, c * xq_chunk:(c + 1) * xq_chunk],
                    xq_d[:, c * xq_chunk:(c + 1) * xq_chunk],
                )
            xqv = xq.rearrange("k (ich q b) -> k ich q b", ich=ICH, q=Q)
            xt = big.tile([128, I * Q], BF16, tag="xt")              # 64K/part
            xt_chunk = I * Q // 4
            for c in range(4):
                nc.sync.dma_start(
                    xt[:, c * xt_chunk:(c + 1) * xt_chunk],
                    xt_d[:, c * xt_chunk:(c + 1) * xt_chunk],
                )

            identity = big.tile([128, 128], F32, tag="ident")
            make_identity(nc, identity[:])
            identb = big.tile([128, 128], BF16, tag="identb")
            make_identity(nc, identb[:])
            eps_t = big.tile([128, 1], F32, tag="eps")
            nc.vector.memset(eps_t[:], EPS)

            # F[ip, (ich, j, b)] bf16: running c (up to global normalizer)
            f_sb = big.tile([128, ICH * JL * B], BF16, tag="f")      # 8K/part
            f_v = f_sb.rearrange("k (ich j b) -> k ich j b", ich=ICH, j=JL)

            # warmup collective to absorb core-start skew
            wu_s = small.tile([128, 8], F32, tag="wu")
            nc.gpsimd.memset(wu_s[:], 0.0)
            wu_i = dram.tile([128, 8], F32, tag="wu_i")
            wu_o = dram.tile([128, 8], F32, tag="wu_o")
            nc.gpsimd.dma_start(wu_i[:], wu_s[:])
            nc.gpsimd.collective_compute(
                "AllReduce", mybir.AluOpType.add,
                replica_groups=[list(range(N_CORES))],
                ins=[wu_i.opt()], outs=[wu_o.opt()],
            )

            # ---- S-pass: vT[(j,p), b] = sum_{i,q} W x ---------------
            with tc.tile_pool(name="ps_s", bufs=1, space="PSUM") as ps_s, \
                 tc.tile_pool(name="ps_st", bufs=1, space="PSUM") as ps_st:
                s_ps = ps_s.tile([128, B], F32, tag="s_ps")
                n_mm = ICH * Q
                k = 0
                for ich in range(ICH):
                    for q in range(Q):
                        nc.tensor.matmul(
                            s_ps[:],
                            wnv[:, ich, q, :, :],       # lhsT [128, (j p)]
                            xqv[:, ich, q, :],          # rhs  [128, 64]
                            start=(k == 0), stop=(k == n_mm - 1),
                        )
                        k += 1
                v_sb = _vT_to_v(nc, small, ps_st, s_ps, identity, scale=1.0 / J)
                _squash(nc, small, v_sb, eps_t)
                vbds = _build_vbd(nc, small, ps_st, v_sb, identity)

            # ---- 2 routing iterations -------------------------------
            for it in range(2):
                first = it == 0
                # b-pass: Delta[b,j,i] via t = vbd^T Wt, u = t*x, tree over q
                cc_pend = [None, None]
                with tc.tile_pool(name=f"ps_b{it}", bufs=3, space="PSUM") as ps_b, \
                     tc.tile_pool(name=f"ps_bt{it}", bufs=2, space="PSUM") as ps_bt:
                    for g in range(ICH):
                        wt_s = wts.tile([128, 128 * Q], BF16, tag="wt_s")
                        nc.sync.dma_start(
                            wt_s[:], wt_d[:, g * 128 * Q:(g + 1) * 128 * Q]
                        )
                        xoff = g * 2048
                        for pair in range(2):
                            d_ps = ps_bt.tile(
                                [128, 128], BF16, tag="d_ps",
                                name=f"d_ps{it}_{g}_{pair}",
                            )
                            for half in range(2):
                                t_ps = ps_b.tile(
                                    [128, 1024], F32, tag="t_ps",
                                    name=f"t_ps{it}_{g}_{pair}_{half}",
                                )
                                for m in range(2):
                                    off = half * 1024 + m * 512
                                    nc.tensor.matmul(
                                        t_ps[:, m * 512:(m + 1) * 512],
                                        vbds[pair][:],
                                        wt_s[:, off:off + 512],
                                        start=True, stop=True,
                                    )
                                ts = tstr.tile(
                                    [128, 1024], BF16, tag="ts",
                                    name=f"ts{it}_{g}_{pair}_{half}",
                                )
                                xsl = xt[:, xoff + half * 1024:
                                         xoff + half * 1024 + 1024]
                                if pair * 2 + half < EVAC_OF_4:
                                    nc.scalar.copy(ts[:], t_ps[:])
                                    nc.vector.tensor_mul(ts[:], ts[:], xsl)
                                else:
                                    nc.vector.tensor_mul(ts[:], t_ps[:], xsl)
                                # tree over q within the half: 1024->512->256
                                nc.vector.tensor_add(
                                    ts[:, 0:512], ts[:, 0:512], ts[:, 512:1024]
                                )
                                nc.vector.tensor_add(
                                    ts[:, 0:256], ts[:, 0:256], ts[:, 256:512]
                                )
                                if TRANSPOSE_ACC:
                                    # last level folded into PE: d_ps += ts^T
                                    nc.tensor.matmul(
                                        d_ps[:], ts[:, 0:128], identb[:],
                                        is_transpose=True,
                                        start=(half == 0), stop=False,
                                        skip_group_check=True,
                                    )
                                    nc.tensor.matmul(
                                        d_ps[:], ts[:, 128:256], identb[:],
                                        is_transpose=True,
                                        start=False, stop=(half == 1),
                                        skip_group_check=True,
                                    )
                                else:
                                    nc.vector.tensor_add(
                                        ts[:, 0:128], ts[:, 0:128],
                                        ts[:, 128:256],
                                    )
                                    if half == 0:
                                        hold = estr.tile(
                                            [128, 128], BF16, tag="hold",
                                            name=f"hold{it}_{g}_{pair}",
                                        )
                                        nc.vector.tensor_copy(
                                            hold[:], ts[:, 0:128]
                                        )
                                    else:
                                        nc.vector.tensor_add(
                                            ts[:, 0:128], ts[:, 0:128], hold[:]
                                        )
                                        nc.tensor.transpose(
                                            d_ps[:], ts[:, 0:128], identb[:]
                                        )
                            off = (g * JL + pair * 2) * B
                            dst = f_sb[:, off:off + 2 * B]
                            if first:
                                nc.scalar.activation(dst, d_ps[:], AF.Exp)
                            else:
                                ex = estr.tile(
                                    [128, 128], BF16, tag="ex",
                                    name=f"ex{it}_{g}_{pair}",
                                )
                                nc.scalar.activation(ex[:], d_ps[:], AF.Exp)
                                nc.vector.tensor_mul(dst, dst, ex[:])
                        if g == 7 or g == ICH - 1:
                            h = 0 if g == 7 else 1
                            sl = slice(h * 8, h * 8 + 8)
                            zph = small.tile(
                                [128, 8 * B], BF16, tag=f"zp{h}",
                                name=f"zp{it}_{h}",
                            )
                            zpv = zph.rearrange("k (ic b) -> k ic b", ic=8)
                            nc.vector.tensor_add(
                                zpv[:], f_v[:, sl, 0, :], f_v[:, sl, 1, :]
                            )
                            for j in range(2, JL):
                                nc.vector.tensor_add(
                                    zpv[:], zpv[:], f_v[:, sl, j, :]
                                )
                            cc_i = dram.tile(
                                [128, 8 * B], BF16, tag=f"cc_i{h}",
                                name=f"cci{it}_{h}",
                            )
                            cc_o = dram.tile(
                                [128, 8 * B], BF16, tag=f"cc_o{h}",
                                name=f"cco{it}_{h}",
                            )
                            nc.gpsimd.dma_start(cc_i[:], zph[:])
                            nc.gpsimd.collective_compute(
                                "AllReduce", mybir.AluOpType.add,
                                replica_groups=[list(range(N_CORES))],
                                ins=[cc_i.opt()], outs=[cc_o.opt()],
                            )
                            cc_pend[h] = cc_o

                # v-pass: vT[(j,p), b] = sum_{i,q} W (F x), col-tiled over j
                with tc.tile_pool(name=f"ps_v{it}", bufs=1, space="PSUM") as ps_v, \
                     tc.tile_pool(name=f"ps_vt{it}", bufs=2, space="PSUM") as ps_vt:
                    vT_ps = ps_v.tile([128, B], F32, tag="vT_ps")
                    for h in range(2):
                        sl = slice(h * 8, h * 8 + 8)
                        zh = small.tile(
                            [128, 8 * B], BF16, tag=f"z{h}", name=f"z{it}_{h}"
                        )
                        nc.sync.dma_start(zh[:], cc_pend[h][:])
                        with nc.allow_low_precision(
                            reason="softmax normalizer; tol 2e-2"
                        ):
                            nc.vector.reciprocal(zh[:], zh[:])
                        zrv = zh.rearrange("k (ic b) -> k ic b", ic=8)
                        for j in range(JL):
                            nc.vector.tensor_mul(
                                f_v[:, sl, j, :], f_v[:, sl, j, :], zrv[:]
                            )
                    for ich in range(ICH):
                        ys = []
                        for j in range(JL):
                            y = ytile.tile(
                                [128, Q * B], BF16, tag="y",
                                name=f"y{it}_{ich}_{j}",
                            )
                            yv = y.rearrange("k (q b) -> k q b", q=Q)
                            cb = (
                                f_v[:, ich, j, :]
                                .unsqueeze(1).broadcast_to([128, Q, B])
                            )
                            nc.vector.tensor_mul(
                                yv[:], xqv[:, ich, :, :], cb[:]
                            )
                            ys.append(yv)
                        for q in range(Q):
                            for j in range(JL):
                                nc.tensor.matmul(
                                    vT_ps[j * P:(j + 1) * P, :],
                                    wnv[:, ich, q, j, :],
                                    ys[j][:, q, :],
                                    start=(ich == 0 and q == 0),
                                    stop=(ich == ICH - 1 and q == Q - 1),
                                    tile_position=(0, j * P),
                                )
                    v_sb = _vT_to_v(nc, small, ps_vt, vT_ps, identity)
                    _squash(nc, small, v_sb, eps_t)
                    if it == 0:
                        vbds = _build_vbd(nc, small, ps_vt, v_sb, identity)
                    else:
                        nc.sync.dma_start(out_d[:], v_sb[:])

    nc.compile()
    _CACHED["nc"] = nc
    return nc


def _prep_inputs(inputs_np, W_np):
    x = np.ascontiguousarray(inputs_np)           # [B, I, Q] f32
    W = np.ascontiguousarray(W_np)                # [J, I, P, Q] f32
    xq = (
        x.reshape(B, ICH, 128, Q).transpose(2, 1, 3, 0)
        .astype(NP_BF16).reshape(128, ICH * Q * B)
    )
    # xt cols ordered (g, q, iw): matches wt streaming windows
    xt_base = (
        x.reshape(B, ICH, 128, Q).transpose(0, 1, 3, 2)   # [b, g, q, iw]
        .astype(NP_BF16).reshape(B, I * Q)
    )
    xt = np.concatenate([xt_base, xt_base], axis=0)
    in_maps = []
    for r in range(N_CORES):
        Wr = W[r * JL:(r + 1) * JL]                       # [4, I, P, Q]
        wn = (
            Wr.reshape(JL, ICH, 128, P, Q).transpose(2, 1, 4, 0, 3)
            .astype(NP_BF16).reshape(128, ICH * Q * JL * P)
        )
        wt = (
            Wr.reshape(JL, ICH, 128, P, Q)
            .transpose(0, 3, 1, 4, 2)                     # [j, p, g, q, iw]
            .astype(NP_BF16).reshape(128, I * Q)
        )
        in_maps.append(
            {
                "wn": np.ascontiguousarray(wn),
                "wt": np.ascontiguousarray(wt),
                "xq": np.ascontiguousarray(xq),
                "xt": np.ascontiguousarray(xt),
            }
        )
    return in_maps


def kernel(inputs, W, _trace=False):
    nc = build_kernel()
    in_maps = _prep_inputs(np.asarray(inputs), np.asarray(W))
    res = run_bass_kernel_spmd(nc, in_maps, list(range(N_CORES)), trace=_trace)
    out = np.concatenate(
        [res.results[r]["o"].reshape(B, JL, P) for r in range(N_CORES)], axis=1
    )
    if _trace:
        kernel.last_exec_ns = res.exec_time_ns
        kernel.last_results = res
    return out.astype(np.float32)
